# revision 3
# baseline (speedup 1.0000x reference)
"""CrossModalAttention kernel for 8 Trainium2 NeuronCores.

Strategy: pure data parallelism — batch B=8, one batch element per core.
Per core, the full attention block runs in fp32r (TF32-like PE precision,
1 cycle/row at free-dim 512) with fp32 accumulation:

  phase 1: Q/K/V projections. Host pre-transposes activations and weights so
           every matmul contraction dim lands on SBUF partitions.
           qT[o, lq], kT[o, lkv] (o = head-major features), v[lkv, o] natural.
  phase 2: per head pair (row/col-packed tile_position matmuls):
           scoresT[lk, lq] = kT_h^T-block @ qT_h; eT = exp(scoresT);
           denom via ones-matmul (broadcast to all partitions); R = 1/denom;
           ctxT[o', lq] col-packed += v-slice^T @ eT, normalized by R on copy;
           attn-weights accumulator accWT[lk, lq] += eT * R (DVE mult, GpSimd add).
  phase 3: attn_out natural [lq, o] = ctxT-slice^T @ wo_t (no transposes needed);
           residual + LayerNorm (bn_stats/bn_aggr) -> out;
           accWT PE-transposed per 128x128 tile, scaled by 1/H -> attn_weights.
"""
import numpy as np

import concourse.bass as bass
import concourse.mybir as mybir
import concourse.tile as tile
from concourse import bacc
from concourse.bass_utils import run_bass_kernel_spmd

F32R = mybir.dt.float32r
F32 = mybir.dt.float32
AOP = mybir.AluOpType
ACTF = mybir.ActivationFunctionType

P = 128
B = 8
LQ = 512
LKV = 1024
D = 1024
H = 16
HD = 64
NC = 8          # cores
OB = D // P     # 8 feature blocks
KC = D // P     # 8 contraction chunks
LKB = LKV // P  # 8 l_k blocks
NSUB = LQ // P  # 4 l_q sub-blocks
LN_EPS = 1e-5

_PROGRAM_CACHE = {}


def _build_program(use_bq, use_bk, use_bv, use_g, use_b):
    nc = bacc.Bacc("TRN2", target_bir_lowering=False)

    xq_t = nc.dram_tensor("xq_t", [D, LQ], F32R, kind="ExternalInput")
    xk_t = nc.dram_tensor("xk_t", [D, LKV], F32R, kind="ExternalInput")
    xv_t = nc.dram_tensor("xv_t", [D, LKV], F32R, kind="ExternalInput")
    xq_nat = nc.dram_tensor("xq_nat", [LQ, D], F32, kind="ExternalInput")
    wq_t = nc.dram_tensor("wq_t", [D, D], F32R, kind="ExternalInput")
    wk_t = nc.dram_tensor("wk_t", [D, D], F32R, kind="ExternalInput")
    wv_t = nc.dram_tensor("wv_t", [D, D], F32R, kind="ExternalInput")
    wo_t = nc.dram_tensor("wo_t", [D, D], F32R, kind="ExternalInput")
    ones_d = nc.dram_tensor("ones_d", [P, P], F32R, kind="ExternalInput")
    ident_d = nc.dram_tensor("ident_d", [P, P], F32, kind="ExternalInput")
    if use_bq:
        bq_d = nc.dram_tensor("bq_v", [D], F32, kind="ExternalInput")
    if use_bk:
        bk_d = nc.dram_tensor("bk_v", [D], F32, kind="ExternalInput")
    if use_bv:
        bv_d = nc.dram_tensor("bv_v", [D], F32, kind="ExternalInput")
    if use_g:
        g_d = nc.dram_tensor("g_v", [D], F32, kind="ExternalInput")
    if use_b:
        b_d = nc.dram_tensor("b_v", [D], F32, kind="ExternalInput")

    out_d = nc.dram_tensor("out", [LQ, D], F32, kind="ExternalOutput")
    aw_d = nc.dram_tensor("attn_w", [LQ, LKV], F32, kind="ExternalOutput")

    def bcast_ap(dram_handle):
        a = dram_handle[:]
        return bass.AP(tensor=a.tensor, offset=a.offset, ap=[[0, P], *a.ap])

    with tile.TileContext(nc) as tc:
        with (
            tc.tile_pool(name="const", bufs=1) as const,
            tc.tile_pool(name="pers", bufs=1) as pers,
        ):
            ones = const.tile([P, P], F32R, tag="ones")
            nc.sync.dma_start(ones, ones_d[:])
            ident = const.tile([P, P], F32, tag="ident")
            nc.sync.dma_start(ident, ident_d[:])
            eps_t = const.tile([P, 1], F32, tag="eps")
            nc.vector.memset(eps_t, LN_EPS)
            if use_bq:
                bq_sb = const.tile([P, OB], F32, tag="bq")
                nc.sync.dma_start(bq_sb, bq_d.rearrange("(o p) -> p o", p=P))
            if use_bk:
                bk_sb = const.tile([P, OB], F32, tag="bk")
                nc.sync.dma_start(bk_sb, bk_d.rearrange("(o p) -> p o", p=P))
            if use_bv:
                bv_bc = const.tile([P, D], F32, tag="bv")
                nc.sync.dma_start(bv_bc, bcast_ap(bv_d))
            if use_g:
                g_bc = const.tile([P, D], F32, tag="g")
                nc.sync.dma_start(g_bc, bcast_ap(g_d))
            if use_b:
                b_bc = const.tile([P, D], F32, tag="b")
                nc.sync.dma_start(b_bc, bcast_ap(b_d))

            qT = pers.tile([P, OB, LQ], F32R, tag="qT")
            kT = pers.tile([P, OB, LKV], F32R, tag="kT")
            v_sb = pers.tile([P, LKB, D], F32R, tag="v")
            ctxT = pers.tile([P, OB, LQ], F32R, tag="ctxT")

            # ---------------- phase 1: projections ----------------
            xq_re = xq_t.rearrange("(c p) l -> p c l", p=P)
            xk_re = xk_t.rearrange("(c p) l -> p c l", p=P)
            xv_re = xv_t.rearrange("(c p) l -> p c l", p=P)
            wq_re = wq_t.rearrange("(c p) o -> p c o", p=P)
            wk_re = wk_t.rearrange("(c p) o -> p c o", p=P)
            wv_re = wv_t.rearrange("(c p) o -> p c o", p=P)

            with (
                tc.tile_pool(name="ph1x", bufs=3) as ph1x,
                tc.tile_pool(name="ph1w", bufs=3) as ph1w,
                tc.tile_pool(name="ps1", bufs=8, space="PSUM") as ps1,
            ):
                # Q projection: qT[o, lq]
                qps = [ps1.tile([P, LQ], F32, tag="ps1", name=f"qps{_ob}") for _ob in range(OB)]
                for kc in range(KC):
                    xch = ph1x.tile([P, LQ], F32R, tag="xch")
                    nc.sync.dma_start(xch, xq_re[:, kc, :])
                    wch = ph1w.tile([P, D], F32R, tag="wch")
                    nc.sync.dma_start(wch, wq_re[:, kc, :])
                    for ob in range(OB):
                        nc.tensor.matmul(qps[ob], wch[:, ob * P:(ob + 1) * P], xch,
                                         start=(kc == 0), stop=(kc == KC - 1))
                for ob in range(OB):
                    if use_bq:
                        nc.vector.tensor_scalar_add(qT[:, ob, :], qps[ob],
                                                    bq_sb[:, ob:ob + 1])
                    else:
                        nc.vector.tensor_copy(qT[:, ob, :], qps[ob])

                # K projection: kT[o, lkv]
                for half in range(2):
                    hsl = slice(half * LQ, (half + 1) * LQ)
                    kps = [ps1.tile([P, LQ], F32, tag="ps1", name=f"kps{half}_{_ob}") for _ob in range(OB)]
                    for kc in range(KC):
                        xch = ph1x.tile([P, LQ], F32R, tag="xch")
                        nc.sync.dma_start(xch, xk_re[:, kc, hsl])
                        wch = ph1w.tile([P, D], F32R, tag="wch")
                        nc.sync.dma_start(wch, wk_re[:, kc, :])
                        for ob in range(OB):
                            nc.tensor.matmul(kps[ob], wch[:, ob * P:(ob + 1) * P], xch,
                                             start=(kc == 0), stop=(kc == KC - 1))
                    for ob in range(OB):
                        if use_bk:
                            nc.vector.tensor_scalar_add(kT[:, ob, hsl], kps[ob],
                                                        bk_sb[:, ob:ob + 1])
                        else:
                            nc.vector.tensor_copy(kT[:, ob, hsl], kps[ob])

                # V projection (natural layout): v[lkv, o]
                for half in range(2):
                    hsl = slice(half * LQ, (half + 1) * LQ)
                    vps = [ps1.tile([P, LQ], F32, tag="ps1", name=f"vps{half}_{_vb}") for _vb in range(LKB)]
                    for kc in range(KC):
                        xch2 = ph1x.tile([P, LKV], F32R, tag="xchv")
                        nc.sync.dma_start(xch2, xv_re[:, kc, :])
                        wch2 = ph1w.tile([P, LQ], F32R, tag="wchv")
                        nc.sync.dma_start(wch2, wv_re[:, kc, hsl])
                        for vb in range(LKB):
                            nc.tensor.matmul(vps[vb], xch2[:, vb * P:(vb + 1) * P], wch2,
                                             start=(kc == 0), stop=(kc == KC - 1))
                    for vb in range(LKB):
                        if use_bv:
                            nc.vector.tensor_tensor(v_sb[:, vb, hsl], vps[vb],
                                                    bv_bc[:, hsl], AOP.add)
                        else:
                            nc.vector.tensor_copy(v_sb[:, vb, hsl], vps[vb])

            # ---------------- phase 2: attention (per head pair) ----------------
            with (
                tc.tile_pool(name="accp", bufs=1) as accp,
                tc.tile_pool(name="wop", bufs=1) as wop,
            ):
                accWT = accp.tile([P, LKB, LQ], F32, tag="accWT")
                wo_sb = wop.tile([P, KC, D], F32R, tag="wo")
                nc.sync.dma_start(wo_sb, wo_t.rearrange("(c p) o -> p c o", p=P))

                with (
                    tc.tile_pool(name="eTp", bufs=2) as eTp,
                    tc.tile_pool(name="Rp", bufs=4) as Rp,
                    tc.tile_pool(name="wtmp", bufs=4) as wtmpp,
                    tc.tile_pool(name="ps_sT", bufs=3, space="PSUM") as sT_ps,
                    tc.tile_pool(name="ps_d", bufs=2, space="PSUM") as d_ps,
                    tc.tile_pool(name="ps_ctx", bufs=2, space="PSUM") as ctx_ps,
                ):
                    for j in range(H // 2):
                        eTa = eTp.tile([P, LKB, LQ], F32R, tag="eT")
                        eTb = eTp.tile([P, LKB, LQ], F32R, tag="eT")
                        eTs = (eTa, eTb)
                        # scoresT + exp, row-packed head pair
                        for kc in range(LKB):
                            for t in (0, 1):
                                rows = slice(HD * t, HD * (t + 1))
                                ps = sT_ps.tile([P, LQ], F32, tag="sT")
                                nc.tensor.matmul(
                                    ps,
                                    kT[rows, j, kc * P:(kc + 1) * P],
                                    qT[rows, j, :],
                                    start=True, stop=True,
                                    tile_position=(HD * t, 0),
                                )
                                nc.scalar.activation(out=eTs[t][:, kc, :], in_=ps,
                                                     func=ACTF.Exp)
                        # denominators (broadcast over partitions) + reciprocal
                        Rs = []
                        for t in (0, 1):
                            dps = d_ps.tile([P, LQ], F32, tag="dps")
                            for kc in range(LKB):
                                nc.tensor.matmul(dps, ones, eTs[t][:, kc, :],
                                                 start=(kc == 0), stop=(kc == LKB - 1))
                            R = Rp.tile([P, LQ], F32, tag="R")
                            nc.vector.reciprocal(R, dps)
                            Rs.append(R)
                        # ctx per head (M=64; odd head partition-shifted via DMA)
                        for t in (0, 1):
                            cps = ctx_ps.tile([P, LQ], F32, tag="cps",
                                              name=f"cps{j}_{t}")
                            h = 2 * j + t
                            for kc in range(LKB):
                                nc.tensor.matmul(
                                    cps[0:HD, :],
                                    v_sb[:, kc, h * HD:(h + 1) * HD],
                                    eTs[t][:, kc, :],
                                    start=(kc == 0), stop=(kc == LKB - 1),
                                )
                            if t == 0:
                                nc.vector.tensor_tensor(ctxT[0:HD, j, :], cps[0:HD, :],
                                                        Rs[0][0:HD, :], AOP.mult)
                            else:
                                cstage = Rp.tile([HD, LQ], F32R, tag="cst")
                                nc.vector.tensor_tensor(cstage, cps[0:HD, :],
                                                        Rs[1][0:HD, :], AOP.mult)
                                nc.sync.dma_start(ctxT[HD:P, j, :], cstage)
                        # attn-weights accumulation accWT += eT * R
                        for t in (0, 1):
                            h = 2 * j + t
                            for kc in range(LKB):
                                if h == 0:
                                    nc.vector.tensor_tensor(
                                        accWT[:, kc, :],
                                        eTs[t][:, kc, :].bitcast(F32), Rs[t], AOP.mult)
                                else:
                                    tmp = wtmpp.tile([P, LQ], F32, tag="wtmp")
                                    nc.vector.tensor_tensor(
                                        tmp, eTs[t][:, kc, :].bitcast(F32), Rs[t],
                                        AOP.mult)
                                    nc.gpsimd.tensor_tensor(
                                        accWT[:, kc, :], accWT[:, kc, :], tmp, AOP.add)

                # ---------------- phase 3: out-proj + LN + weight transpose ----
                with (
                    tc.tile_pool(name="ph3", bufs=2) as ph3,
                    tc.tile_pool(name="qnp", bufs=1) as qnp,
                    tc.tile_pool(name="lnp", bufs=4) as lnp,
                    tc.tile_pool(name="ps_ao", bufs=2, space="PSUM") as ao_ps,
                    tc.tile_pool(name="ps_tr", bufs=2, space="PSUM") as tr_ps,
                ):
                    xq_nat_sb = qnp.tile([P, NSUB, D], F32, tag="qnat")
                    nc.sync.dma_start(xq_nat_sb,
                                      xq_nat.rearrange("(s p) o -> p s o", p=P))
                    for sub in range(NSUB):
                        ssl = slice(sub * P, (sub + 1) * P)
                        xs = ph3.tile([P, D], F32, tag="xs")
                        for half in range(2):
                            hsl = slice(half * LQ, (half + 1) * LQ)
                            aps = ao_ps.tile([P, LQ], F32, tag="aps")
                            for oc in range(KC):
                                nc.tensor.matmul(aps, ctxT[:, oc, ssl],
                                                 wo_sb[:, oc, hsl],
                                                 start=(oc == 0), stop=(oc == KC - 1))
                            nc.vector.tensor_tensor(xs[:, hsl], aps,
                                                    xq_nat_sb[:, sub, hsl], AOP.add)
                        # LayerNorm over D
                        stats = lnp.tile([P, 2, 6], F32, tag="stats")
                        xs3 = xs.rearrange("p (s f) -> p s f", s=2)
                        for sgi in range(2):
                            nc.vector.bn_stats(out=stats[:, sgi, :], in_=xs3[:, sgi, :])
                        mv = lnp.tile([P, 2], F32, tag="mv")
                        nc.vector.bn_aggr(out=mv, in_=stats)
                        std = lnp.tile([P, 1], F32, tag="std")
                        nc.scalar.activation(out=std, in_=mv[:, 1:2], func=ACTF.Sqrt,
                                             bias=eps_t)
                        nc.vector.reciprocal(std, std)
                        nc.vector.tensor_scalar(out=xs, in0=xs, scalar1=mv[:, 0:1],
                                                scalar2=std, op0=AOP.subtract,
                                                op1=AOP.mult)
                        if use_g:
                            nc.vector.tensor_tensor(xs, xs, g_bc, AOP.mult)
                        if use_b:
                            nc.vector.tensor_tensor(xs, xs, b_bc, AOP.add)
                        nc.sync.dma_start(out_d[ssl, :], xs)

                        # attn_weights natural tile via PE transpose, scaled 1/H
                        wn = ph3.tile([P, LKV], F32, tag="wn")
                        for kc in range(LKB):
                            tp = tr_ps.tile([P, P], F32, tag="tp")
                            nc.tensor.transpose(tp, accWT[:, kc, ssl], ident)
                            nc.vector.tensor_scalar_mul(wn[:, kc * P:(kc + 1) * P],
                                                        tp, 1.0 / H)
                        nc.sync.dma_start(aw_d[ssl, :], wn)

    nc.compile()
    return nc


def kernel(query, key, value, in_proj_w, in_proj_b, out_proj_w, out_proj_b,
           ln_gamma, ln_beta):
    query = np.asarray(query, dtype=np.float32)
    key = np.asarray(key, dtype=np.float32)
    value = np.asarray(value, dtype=np.float32)
    in_proj_w = np.asarray(in_proj_w, dtype=np.float32)
    in_proj_b = np.asarray(in_proj_b, dtype=np.float32)
    out_proj_w = np.asarray(out_proj_w, dtype=np.float32)
    out_proj_b = np.asarray(out_proj_b, dtype=np.float32)
    ln_gamma = np.asarray(ln_gamma, dtype=np.float32)
    ln_beta = np.asarray(ln_beta, dtype=np.float32)

    scale = 1.0 / np.sqrt(HD)
    wq, wk, wv = in_proj_w[0:D], in_proj_w[D:2 * D], in_proj_w[2 * D:3 * D]
    bq, bk, bv = in_proj_b[0:D] * scale, in_proj_b[D:2 * D], in_proj_b[2 * D:3 * D]
    bo = out_proj_b

    wq_t = np.ascontiguousarray(wq.T * scale)
    wk_t = np.ascontiguousarray(wk.T)
    wv_t = np.ascontiguousarray(wv.T)
    wo_t = np.ascontiguousarray(out_proj_w.T)

    use_bq = bool(np.any(bq != 0))
    use_bk = bool(np.any(bk != 0))
    use_bv = bool(np.any(bv != 0))
    use_g = bool(np.any(ln_gamma != 1.0))
    use_b = bool(np.any(ln_beta != 0))

    flags = (use_bq, use_bk, use_bv, use_g, use_b)
    if flags not in _PROGRAM_CACHE:
        _PROGRAM_CACHE[flags] = _build_program(*flags)
    nc = _PROGRAM_CACHE[flags]

    ones_np = np.ones((P, P), np.float32)
    ident_np = np.eye(P, dtype=np.float32)

    in_maps = []
    for b in range(B):
        m = dict(
            xq_t=np.ascontiguousarray(query[b].T),
            xk_t=np.ascontiguousarray(key[b].T),
            xv_t=np.ascontiguousarray(value[b].T),
            xq_nat=np.ascontiguousarray(query[b] + bo[None, :]),
            wq_t=wq_t, wk_t=wk_t, wv_t=wv_t, wo_t=wo_t,
            ones_d=ones_np, ident_d=ident_np,
        )
        if use_bq:
            m["bq_v"] = bq
        if use_bk:
            m["bk_v"] = bk
        if use_bv:
            m["bv_v"] = bv
        if use_g:
            m["g_v"] = ln_gamma
        if use_b:
            m["b_v"] = ln_beta
        in_maps.append(m)

    res = run_bass_kernel_spmd(nc, in_maps, list(range(NC)))
    output = np.stack([res.results[c]["out"] for c in range(NC)])
    attn_weights = np.stack([res.results[c]["attn_w"] for c in range(NC)])
    return output, attn_weights


# revision 6
# speedup vs baseline: 1.3642x; 1.3642x over previous
"""CrossModalAttention kernel for 8 Trainium2 NeuronCores.

Strategy: pure data parallelism — batch B=8, one batch element per core.
Per core, the full attention block runs in fp32r (TF32-like PE precision,
1 cycle/row at free-dim 512) with fp32 accumulation:

  phase 1: Q/K/V projections. Host pre-transposes activations and weights so
           every matmul contraction dim lands on SBUF partitions.
           qT[o, lq], kT[o, lkv] (o = head-major features), v[lkv, o] natural.
  phase 2: per head pair (row/col-packed tile_position matmuls):
           scoresT[lk, lq] = kT_h^T-block @ qT_h; eT = exp(scoresT);
           denom via ones-matmul (broadcast to all partitions); R = 1/denom;
           ctxT[o', lq] col-packed += v-slice^T @ eT, normalized by R on copy;
           attn-weights accumulator accWT[lk, lq] += eT * R (DVE mult, GpSimd add).
  phase 3: attn_out natural [lq, o] = ctxT-slice^T @ wo_t (no transposes needed);
           residual + LayerNorm (bn_stats/bn_aggr) -> out;
           accWT PE-transposed per 128x128 tile, scaled by 1/H -> attn_weights.
"""
import numpy as np

import concourse.bass as bass
import concourse.mybir as mybir
import concourse.tile as tile
from concourse import bacc
from concourse.bass_utils import run_bass_kernel_spmd

F32R = mybir.dt.float32r
F32 = mybir.dt.float32
BF16 = mybir.dt.bfloat16
AOP = mybir.AluOpType
ACTF = mybir.ActivationFunctionType

P = 128
B = 8
LQ = 512
LKV = 1024
D = 1024
H = 16
HD = 64
NC = 8          # cores
OB = D // P     # 8 feature blocks
KC = D // P     # 8 contraction chunks
LKB = LKV // P  # 8 l_k blocks
NSUB = LQ // P  # 4 l_q sub-blocks
LN_EPS = 1e-5

_PROGRAM_CACHE = {}


def _build_program(use_bq, use_bk, use_bv, use_g, use_b):
    nc = bacc.Bacc("TRN2", target_bir_lowering=False)

    xq_t = nc.dram_tensor("xq_t", [D, LQ], F32R, kind="ExternalInput")
    xk_t = nc.dram_tensor("xk_t", [D, LKV], F32R, kind="ExternalInput")
    xv_t = nc.dram_tensor("xv_t", [D, LKV], F32R, kind="ExternalInput")
    xq_nat = nc.dram_tensor("xq_nat", [LQ, D], F32, kind="ExternalInput")
    wq_t = nc.dram_tensor("wq_t", [D, D], F32R, kind="ExternalInput")
    wk_t = nc.dram_tensor("wk_t", [D, D], F32R, kind="ExternalInput")
    wv_t = nc.dram_tensor("wv_t", [D, D], F32R, kind="ExternalInput")
    wo_t = nc.dram_tensor("wo_t", [D, D], F32R, kind="ExternalInput")
    ones_d = nc.dram_tensor("ones_d", [P, P], F32R, kind="ExternalInput")
    ident_d = nc.dram_tensor("ident_d", [P, P], F32, kind="ExternalInput")
    identb_d = nc.dram_tensor("identb_d", [P, P], BF16, kind="ExternalInput")
    if use_bq:
        bq_d = nc.dram_tensor("bq_v", [D], F32, kind="ExternalInput")
    if use_bk:
        bk_d = nc.dram_tensor("bk_v", [D], F32, kind="ExternalInput")
    if use_bv:
        bv_d = nc.dram_tensor("bv_v", [D], F32, kind="ExternalInput")
    if use_g:
        g_d = nc.dram_tensor("g_v", [D], F32, kind="ExternalInput")
    if use_b:
        b_d = nc.dram_tensor("b_v", [D], F32, kind="ExternalInput")

    out_d = nc.dram_tensor("out", [LQ, D], F32, kind="ExternalOutput")
    aw_d = nc.dram_tensor("attn_w", [LQ, LKV], F32, kind="ExternalOutput")

    def bcast_ap(dram_handle):
        a = dram_handle[:]
        return bass.AP(tensor=a.tensor, offset=a.offset, ap=[[0, P], *a.ap])

    with tile.TileContext(nc) as tc:
        with (
            tc.tile_pool(name="const", bufs=1) as const,
            tc.tile_pool(name="pers", bufs=1) as pers,
        ):
            ones = const.tile([P, P], F32R, tag="ones")
            nc.sync.dma_start(ones, ones_d[:])
            ident = const.tile([P, P], F32, tag="ident")
            nc.sync.dma_start(ident, ident_d[:])
            identb = const.tile([P, P], BF16, tag="identb")
            nc.sync.dma_start(identb, identb_d[:])
            eps_t = const.tile([P, 1], F32, tag="eps")
            nc.vector.memset(eps_t, LN_EPS)
            if use_bq:
                bq_sb = const.tile([P, OB], F32, tag="bq")
                nc.sync.dma_start(bq_sb, bq_d.rearrange("(o p) -> p o", p=P))
            if use_bk:
                bk_sb = const.tile([P, OB], F32, tag="bk")
                nc.sync.dma_start(bk_sb, bk_d.rearrange("(o p) -> p o", p=P))
            if use_bv:
                bv_bc = const.tile([P, D], F32, tag="bv")
                nc.sync.dma_start(bv_bc, bcast_ap(bv_d))
            if use_g:
                g_bc = const.tile([P, D], F32, tag="g")
                nc.sync.dma_start(g_bc, bcast_ap(g_d))
            if use_b:
                b_bc = const.tile([P, D], F32, tag="b")
                nc.sync.dma_start(b_bc, bcast_ap(b_d))

            qT = pers.tile([P, OB, LQ], F32R, tag="qT")
            kT = pers.tile([P, OB, LKV], F32R, tag="kT")
            v_sb = pers.tile([P, LKB, D], F32R, tag="v")
            ctxT = pers.tile([P, OB, LQ], F32R, tag="ctxT")

            # ---------------- phase 1: projections ----------------
            xq_re = xq_t.rearrange("(c p) l -> p c l", p=P)
            xk_re = xk_t.rearrange("(c p) l -> p c l", p=P)
            xv_re = xv_t.rearrange("(c p) l -> p c l", p=P)
            wq_re = wq_t.rearrange("(c p) o -> p c o", p=P)
            wk_re = wk_t.rearrange("(c p) o -> p c o", p=P)
            wv_re = wv_t.rearrange("(c p) o -> p c o", p=P)

            with (
                tc.tile_pool(name="ph1x", bufs=3) as ph1x,
                tc.tile_pool(name="ph1w", bufs=3) as ph1w,
                tc.tile_pool(name="ps1", bufs=8, space="PSUM") as ps1,
            ):
                # Q projection: qT[o, lq]
                qps = [ps1.tile([P, LQ], F32, tag="ps1", name=f"qps{_ob}") for _ob in range(OB)]
                for kc in range(KC):
                    xch = ph1x.tile([P, LQ], F32R, tag="xch")
                    nc.sync.dma_start(xch, xq_re[:, kc, :])
                    wch = ph1w.tile([P, D], F32R, tag="wch")
                    nc.sync.dma_start(wch, wq_re[:, kc, :])
                    for ob in range(OB):
                        nc.tensor.matmul(qps[ob], wch[:, ob * P:(ob + 1) * P], xch,
                                         start=(kc == 0), stop=(kc == KC - 1))
                for ob in range(OB):
                    if use_bq:
                        nc.vector.tensor_scalar_add(qT[:, ob, :], qps[ob],
                                                    bq_sb[:, ob:ob + 1])
                    else:
                        nc.vector.tensor_copy(qT[:, ob, :], qps[ob])

                # K projection: kT[o, lkv]
                for half in range(2):
                    hsl = slice(half * LQ, (half + 1) * LQ)
                    kps = [ps1.tile([P, LQ], F32, tag="ps1", name=f"kps{half}_{_ob}") for _ob in range(OB)]
                    for kc in range(KC):
                        xch = ph1x.tile([P, LQ], F32R, tag="xch")
                        nc.sync.dma_start(xch, xk_re[:, kc, hsl])
                        wch = ph1w.tile([P, D], F32R, tag="wch")
                        nc.sync.dma_start(wch, wk_re[:, kc, :])
                        for ob in range(OB):
                            nc.tensor.matmul(kps[ob], wch[:, ob * P:(ob + 1) * P], xch,
                                             start=(kc == 0), stop=(kc == KC - 1))
                    for ob in range(OB):
                        if use_bk:
                            nc.vector.tensor_scalar_add(kT[:, ob, hsl], kps[ob],
                                                        bk_sb[:, ob:ob + 1])
                        else:
                            nc.vector.tensor_copy(kT[:, ob, hsl], kps[ob])

                # V projection (natural layout): v[lkv, o]
                for half in range(2):
                    hsl = slice(half * LQ, (half + 1) * LQ)
                    vps = [ps1.tile([P, LQ], F32, tag="ps1", name=f"vps{half}_{_vb}") for _vb in range(LKB)]
                    for kc in range(KC):
                        xch2 = ph1x.tile([P, LKV], F32R, tag="xchv")
                        nc.sync.dma_start(xch2, xv_re[:, kc, :])
                        wch2 = ph1w.tile([P, LQ], F32R, tag="wchv")
                        nc.sync.dma_start(wch2, wv_re[:, kc, hsl])
                        for vb in range(LKB):
                            nc.tensor.matmul(vps[vb], xch2[:, vb * P:(vb + 1) * P], wch2,
                                             start=(kc == 0), stop=(kc == KC - 1))
                    for vb in range(LKB):
                        if use_bv:
                            nc.vector.tensor_tensor(v_sb[:, vb, hsl], vps[vb],
                                                    bv_bc[:, hsl], AOP.add)
                        else:
                            nc.vector.tensor_copy(v_sb[:, vb, hsl], vps[vb])

            # ---------------- phase 2: attention (per head pair) ----------------
            with (
                tc.tile_pool(name="accp", bufs=1) as accp,
                tc.tile_pool(name="wop", bufs=1) as wop,
            ):
                accWT = accp.tile([P, LKB, LQ], BF16, tag="accWT")
                wo_sb = wop.tile([P, KC, D], F32R, tag="wo")
                nc.sync.dma_start(wo_sb, wo_t.rearrange("(c p) o -> p c o", p=P))

                with (
                    tc.tile_pool(name="eTp", bufs=20) as eTp,
                    tc.tile_pool(name="Rp", bufs=4) as Rp,
                    tc.tile_pool(name="wtmp", bufs=4) as wtmpp,
                    tc.tile_pool(name="ps_sT", bufs=4, space="PSUM") as sT_ps,
                    tc.tile_pool(name="ps_d", bufs=2, space="PSUM") as d_ps,
                    tc.tile_pool(name="ps_ctx", bufs=2, space="PSUM") as ctx_ps,
                ):
                    for j in range(H // 2):
                        eTs = [[None] * LKB, [None] * LKB]
                        # scoresT + exp, row-packed head pair
                        for kc in range(LKB):
                            for t in (0, 1):
                                rows = slice(HD * t, HD * (t + 1))
                                ps = sT_ps.tile([P, LQ], F32, tag="sT")
                                nc.tensor.matmul(
                                    ps,
                                    kT[rows, j, kc * P:(kc + 1) * P],
                                    qT[rows, j, :],
                                    start=True, stop=True,
                                    tile_position=(HD * t, 0),
                                )
                                eT_t = eTp.tile([P, LQ], F32R, tag="eT",
                                                name=f"eT{j}_{t}_{kc}")
                                eTs[t][kc] = eT_t
                                nc.scalar.activation(out=eT_t, in_=ps,
                                                     func=ACTF.Exp)
                        # denominators (broadcast over partitions) + reciprocal
                        Rs = []
                        for t in (0, 1):
                            dps = d_ps.tile([P, LQ], F32, tag="dps")
                            for kc in range(LKB):
                                nc.tensor.matmul(dps, ones, eTs[t][kc],
                                                 start=(kc == 0), stop=(kc == LKB - 1))
                            R = Rp.tile([P, LQ], F32, tag="R")
                            nc.vector.reciprocal(R, dps)
                            Rs.append(R)
                        # ctx per head (M=64; odd head partition-shifted via DMA)
                        for t in (0, 1):
                            cps = ctx_ps.tile([P, LQ], F32, tag="cps",
                                              name=f"cps{j}_{t}")
                            h = 2 * j + t
                            for kc in range(LKB):
                                nc.tensor.matmul(
                                    cps[0:HD, :],
                                    v_sb[:, kc, h * HD:(h + 1) * HD],
                                    eTs[t][kc],
                                    start=(kc == 0), stop=(kc == LKB - 1),
                                )
                            if t == 0:
                                nc.vector.tensor_tensor(ctxT[0:HD, j, :], cps[0:HD, :],
                                                        Rs[0][0:HD, :], AOP.mult)
                            else:
                                cstage = Rp.tile([HD, LQ], F32R, tag="cst")
                                nc.vector.tensor_tensor(cstage, cps[0:HD, :],
                                                        Rs[1][0:HD, :], AOP.mult)
                                nc.sync.dma_start(ctxT[HD:P, j, :], cstage)
                        # attn-weights accumulation accWT += eT * R (bf16 acc)
                        for t in (0, 1):
                            h = 2 * j + t
                            for kc in range(LKB):
                                if h == 0:
                                    nc.vector.tensor_tensor(
                                        accWT[:, kc, :],
                                        eTs[t][kc][:].bitcast(F32), Rs[t], AOP.mult)
                                else:
                                    tmp = wtmpp.tile([P, LQ], BF16, tag="wtmp")
                                    nc.vector.tensor_tensor(
                                        tmp, eTs[t][kc][:].bitcast(F32), Rs[t],
                                        AOP.mult)
                                    nc.gpsimd.tensor_tensor(
                                        accWT[:, kc, :], accWT[:, kc, :], tmp, AOP.add)

                # ---------------- phase 3: out-proj + LN + weight transpose ----
                with (
                    tc.tile_pool(name="ph3", bufs=2) as ph3,
                    tc.tile_pool(name="qnp", bufs=1) as qnp,
                    tc.tile_pool(name="lnp", bufs=4) as lnp,
                    tc.tile_pool(name="ps_ao", bufs=2, space="PSUM") as ao_ps,
                    tc.tile_pool(name="ps_tr", bufs=2, space="PSUM") as tr_ps,
                ):
                    xq_nat_sb = qnp.tile([P, NSUB, D], F32, tag="qnat")
                    nc.sync.dma_start(xq_nat_sb,
                                      xq_nat.rearrange("(s p) o -> p s o", p=P))
                    for sub in range(NSUB):
                        ssl = slice(sub * P, (sub + 1) * P)
                        xs = ph3.tile([P, D], F32, tag="xs")
                        for half in range(2):
                            hsl = slice(half * LQ, (half + 1) * LQ)
                            aps = ao_ps.tile([P, LQ], F32, tag="aps")
                            for oc in range(KC):
                                nc.tensor.matmul(aps, ctxT[:, oc, ssl],
                                                 wo_sb[:, oc, hsl],
                                                 start=(oc == 0), stop=(oc == KC - 1))
                            nc.vector.tensor_tensor(xs[:, hsl], aps,
                                                    xq_nat_sb[:, sub, hsl], AOP.add)
                        # LayerNorm over D
                        stats = lnp.tile([P, 2, 6], F32, tag="stats")
                        xs3 = xs.rearrange("p (s f) -> p s f", s=2)
                        for sgi in range(2):
                            nc.vector.bn_stats(out=stats[:, sgi, :], in_=xs3[:, sgi, :])
                        mv = lnp.tile([P, 2], F32, tag="mv")
                        nc.vector.bn_aggr(out=mv, in_=stats)
                        std = lnp.tile([P, 1], F32, tag="std")
                        nc.scalar.activation(out=std, in_=mv[:, 1:2], func=ACTF.Sqrt,
                                             bias=eps_t)
                        nc.vector.reciprocal(std, std)
                        nc.vector.tensor_scalar(out=xs, in0=xs, scalar1=mv[:, 0:1],
                                                scalar2=std, op0=AOP.subtract,
                                                op1=AOP.mult)
                        if use_g:
                            nc.vector.tensor_tensor(xs, xs, g_bc, AOP.mult)
                        if use_b:
                            nc.vector.tensor_tensor(xs, xs, b_bc, AOP.add)
                        nc.sync.dma_start(out_d[ssl, :], xs)

                        # attn_weights natural tile via PE transpose, scaled 1/H
                        wn = ph3.tile([P, LKV], F32, tag="wn")
                        for kc in range(LKB):
                            tp = tr_ps.tile([P, P], BF16, tag="tp")
                            nc.tensor.transpose(tp, accWT[:, kc, ssl], identb)
                            nc.vector.tensor_scalar_mul(wn[:, kc * P:(kc + 1) * P],
                                                        tp, 1.0 / H)
                        nc.sync.dma_start(aw_d[ssl, :], wn)

    nc.compile()
    return nc


def make_in_maps(query, key, value, in_proj_w, in_proj_b, out_proj_w, out_proj_b,
                 ln_gamma, ln_beta):
    """Host-side sharding/layout prep. Returns (flags, in_maps)."""
    import ml_dtypes
    query = np.asarray(query, dtype=np.float32)
    key = np.asarray(key, dtype=np.float32)
    value = np.asarray(value, dtype=np.float32)
    in_proj_w = np.asarray(in_proj_w, dtype=np.float32)
    in_proj_b = np.asarray(in_proj_b, dtype=np.float32)
    out_proj_w = np.asarray(out_proj_w, dtype=np.float32)
    out_proj_b = np.asarray(out_proj_b, dtype=np.float32)
    ln_gamma = np.asarray(ln_gamma, dtype=np.float32)
    ln_beta = np.asarray(ln_beta, dtype=np.float32)

    scale = 1.0 / np.sqrt(HD)
    wq, wk, wv = in_proj_w[0:D], in_proj_w[D:2 * D], in_proj_w[2 * D:3 * D]
    bq, bk, bv = in_proj_b[0:D] * scale, in_proj_b[D:2 * D], in_proj_b[2 * D:3 * D]
    bo = out_proj_b

    wq_t = np.ascontiguousarray(wq.T * scale)
    wk_t = np.ascontiguousarray(wk.T)
    wv_t = np.ascontiguousarray(wv.T)
    wo_t = np.ascontiguousarray(out_proj_w.T)

    use_bq = bool(np.any(bq != 0))
    use_bk = bool(np.any(bk != 0))
    use_bv = bool(np.any(bv != 0))
    use_g = bool(np.any(ln_gamma != 1.0))
    use_b = bool(np.any(ln_beta != 0))
    flags = (use_bq, use_bk, use_bv, use_g, use_b)

    ones_np = np.ones((P, P), np.float32)
    ident_np = np.eye(P, dtype=np.float32)
    identb_np = np.eye(P, dtype=ml_dtypes.bfloat16)

    in_maps = []
    for b in range(B):
        m = dict(
            xq_t=np.ascontiguousarray(query[b].T),
            xk_t=np.ascontiguousarray(key[b].T),
            xv_t=np.ascontiguousarray(value[b].T),
            xq_nat=np.ascontiguousarray(query[b] + bo[None, :]),
            wq_t=wq_t, wk_t=wk_t, wv_t=wv_t, wo_t=wo_t,
            ones_d=ones_np, ident_d=ident_np, identb_d=identb_np,
        )
        if use_bq:
            m["bq_v"] = bq
        if use_bk:
            m["bk_v"] = bk
        if use_bv:
            m["bv_v"] = bv
        if use_g:
            m["g_v"] = ln_gamma
        if use_b:
            m["b_v"] = ln_beta
        in_maps.append(m)
    return flags, in_maps


def kernel(query, key, value, in_proj_w, in_proj_b, out_proj_w, out_proj_b,
           ln_gamma, ln_beta):
    flags, in_maps = make_in_maps(query, key, value, in_proj_w, in_proj_b,
                                  out_proj_w, out_proj_b, ln_gamma, ln_beta)
    if flags not in _PROGRAM_CACHE:
        _PROGRAM_CACHE[flags] = _build_program(*flags)
    nc = _PROGRAM_CACHE[flags]
    res = run_bass_kernel_spmd(nc, in_maps, list(range(NC)))
    output = np.stack([res.results[c]["out"] for c in range(NC)])
    attn_weights = np.stack([res.results[c]["attn_w"] for c in range(NC)])
    return output, attn_weights
